# revision 11
# baseline (speedup 1.0000x reference)
"""Trainium2 Bass kernel for DirectionalConvLayer.

Problem: 4 directional 3-tap convs over [256, 256, 15, 15] fp32 images, one
input per direction (horizontal / vertical / main-diagonal / anti-diagonal
taps), shared weight [256, 256, 3] and bias [256].

Strategy: every direction is a 1-D 3-tap conv along its set of lines
(rows / columns / diagonals / anti-diagonals) with a dense 256x256 channel
mix per tap. On the host, ALL lines of ALL four inputs are packed into one
flat stream (single zero separator between consecutive lines), split evenly
across 8 cores at line boundaries. The device kernel is direction-agnostic:
a pure 3-tap conv along the flat axis — accumulating matmuls against
+/-1-shifted views of the stream, contraction over C_in in two 128-chunks.

Transfers and matmul operands are float16 (10-bit mantissa, ~ the tensor
engine's own fp32r precision) at full 1-cycle/row PE rate; PSUM accumulates
in fp32. Per core: 62 free-tiles of 512 cols; per tile x 2 cout-chunks:
6 accumulating matmuls (3 taps x 2 cin-chunks), then a vector-engine
PSUM->SBUF cast and DMA out. Bias is added on the host during unpacking.
"""
from contextlib import ExitStack

import numpy as np

import concourse.bass as bass
import concourse.tile as tile
from concourse import mybir
from concourse.bass_utils import run_bass_kernel_spmd

P = 128
FT = 512
NTILE = 62
CORE_COLS = NTILE * FT     # 31744 cols per core
H = W = 15
NCORE = 8

MM_DT = mybir.dt.float16
MM_NP = np.float16


def _build_lines(d):
    if d == 0:
        return [[(i, j) for j in range(W)] for i in range(H)]
    if d == 1:
        return [[(i, j) for i in range(H)] for j in range(W)]
    if d == 2:
        return [
            [(i, i - k) for i in range(max(0, k), min(H, H + k))]
            for k in range(-(W - 1), W)
        ]
    return [
        [(i, s - i) for i in range(max(0, s - (W - 1)), min(H, s + 1))]
        for s in range(H + W - 1)
    ]


def _build_stream_map():
    """Greedy-pack every (direction, image, line) into NCORE x CORE_COLS.
    colmap[d, b, i*W+j] = core * CORE_COLS + local_col. Consecutive cells of
    a line are adjacent; one zero separator between lines; core slices start
    at line starts, so the +/-1 conv taps only ever cross into zeros."""
    colmap = np.full((4, 256, H * W), -1, np.int64)
    core, col = 0, 0
    for d in range(4):
        lines = _build_lines(d)
        for b in range(256):
            for ln in lines:
                ll = len(ln)
                if col + ll > CORE_COLS:
                    core += 1
                    col = 0
                    assert core < NCORE, "stream overflow"
                for i, (r, c) in enumerate(ln):
                    colmap[d, b, r * W + c] = core * CORE_COLS + col + i
                col += ll + 1
    assert (colmap >= 0).all()
    return colmap


_COLMAP = _build_stream_map()


def _split_drain_waits(nc, max_waits=1):
    """Workaround for this walrus build's 'Too many sync wait commands' limit
    (1 sync wait per instruction): hoist excess sem-waits onto nop
    instructions inserted right before the instruction on the same engine.
    Sequential waits on one engine queue are equivalent to multiple waits on
    one instruction."""
    n = 0
    for fn in nc.m.functions:
        for bb in fn.blocks:
            insts = bb.instructions
            i = 0
            while i < len(insts):
                inst = insts[i]
                si = inst.sync_info
                if si is not None and si.on_wait and len(si.on_wait) > max_waits:
                    extra = list(si.on_wait)[max_waits:]
                    si.on_wait = list(si.on_wait)[:max_waits]
                    for wt in extra:
                        nop = mybir.InstNoOp(
                            name=f"I-waitsplit-{n}",
                            engine=inst.engine,
                            sync_info=mybir.SyncInfo(on_wait=[wt], on_update=[]),
                        )
                        nc.register_instruction(nop)
                        n += 1
                        insts.insert(i, nop)
                        i += 1
                i += 1
    return n


def build_program():
    nc = bass.Bass("TRN2", target_bir_lowering=False, debug=False, num_devices=8)
    xin = nc.dram_tensor(
        "xin", [P, 2, CORE_COLS + 2], MM_DT, kind="ExternalInput"
    ).ap()
    wts = nc.dram_tensor(
        "wts", [P, 2, 3, 2, 128], MM_DT, kind="ExternalInput"
    ).ap()
    yout = nc.dram_tensor(
        "yout", [P, 2, CORE_COLS], MM_DT, kind="ExternalOutput"
    ).ap()

    # DMA unit sizes in free-tiles: small prologue units so the PE starts
    # quickly, large middle units for DMA efficiency, small epilogue units to
    # shorten the tail.
    units = [1, 1, 2] + [4] * 13 + [2, 2, 1, 1]
    assert sum(units) == NTILE

    with tile.TileContext(nc) as tc, ExitStack() as ctx:
        cpool = ctx.enter_context(tc.tile_pool(name="const", bufs=1))
        xpool = ctx.enter_context(tc.tile_pool(name="x", bufs=3))
        ypool = ctx.enter_context(tc.tile_pool(name="y", bufs=3))
        ppool = ctx.enter_context(tc.tile_pool(name="ps", bufs=8, space="PSUM"))

        wt0 = cpool.tile([P, 3, 2, 128], MM_DT)
        nc.gpsimd.dma_start(wt0[:], wts[:, 0])
        wt1 = cpool.tile([P, 3, 2, 128], MM_DT)
        nc.gpsimd.dma_start(wt1[:], wts[:, 1])
        wthalf = (wt0, wt1)

        tile0 = 0
        for u in units:
            ucol = u * FT
            base = tile0 * FT
            xt = xpool.tile([P, 2, ucol + 2], MM_DT)
            nc.sync.dma_start(xt[:], xin[:, :, base : base + ucol + 2])
            yt = ypool.tile([P, 2, ucol], MM_DT)
            for f in range(u):
                for o in range(2):
                    ps = ppool.tile([P, FT], mybir.dt.float32)
                    g = 0
                    for t in range(3):
                        for k in range(2):
                            lhsT = wthalf[o][:, t, k, :]
                            rhs = xt[:, k, f * FT + t : f * FT + t + FT]
                            nc.tensor.matmul(
                                ps[:], lhsT, rhs, start=(g == 0), stop=(g == 5)
                            )
                            g += 1
                    nc.vector.tensor_copy(yt[:, o, f * FT : (f + 1) * FT], ps[:])
            nc.sync.dma_start(yout[:, :, base : base + ucol], yt[:])
            tile0 += u
    _split_drain_waits(nc)
    return nc


def pack_inputs(xs, weight):
    """xs: list of 4 arrays [256, 256, 15, 15] fp32. in_maps for cores 0-7."""
    # w_dev[p, o2, t, k, m] = weight[o2*128+m, k*128+p, t]
    w_dev = np.ascontiguousarray(
        weight.reshape(2, P, 2, P, 3)       # [o2, m, k, p, t]
        .transpose(3, 0, 4, 2, 1)           # [p, o2, t, k, m]
    ).astype(MM_NP)

    C = 256
    xflat = np.zeros((C, NCORE * CORE_COLS), MM_NP)
    for d in range(4):
        xflat[:, _COLMAP[d].reshape(-1)] = (
            xs[d].transpose(1, 0, 2, 3).reshape(C, -1).astype(MM_NP)
        )

    in_maps = []
    for core in range(NCORE):
        seg = xflat[:, core * CORE_COLS : (core + 1) * CORE_COLS]
        xin_np = np.zeros((P, 2, CORE_COLS + 2), MM_NP)
        xin_np[:, 0, 1 : CORE_COLS + 1] = seg[:P]
        xin_np[:, 1, 1 : CORE_COLS + 1] = seg[P:]
        in_maps.append({"xin": xin_np, "wts": w_dev})
    return in_maps


def unpack_outputs(results, bias):
    O = 256
    yflat = np.empty((O, NCORE * CORE_COLS), np.float32)
    for core in range(NCORE):
        yo = np.asarray(results[core]["yout"])        # [128, 2, CORE_COLS] fp16
        yflat[:, core * CORE_COLS : (core + 1) * CORE_COLS] = (
            yo.transpose(1, 0, 2).reshape(O, CORE_COLS).astype(np.float32)
        )
    outs = []
    b = bias[None, :, None].astype(np.float32)
    for d in range(4):
        yd = yflat[:, _COLMAP[d].reshape(-1)].reshape(O, 256, H * W)
        yd = yd.transpose(1, 0, 2) + b
        outs.append(np.ascontiguousarray(yd.reshape(256, 256, H, W)))
    return tuple(outs)


def kernel(x0, x1, x2, x3, weight, bias):
    xs = [np.ascontiguousarray(np.asarray(a, dtype=np.float32)) for a in (x0, x1, x2, x3)]
    weight = np.asarray(weight, dtype=np.float32)
    bias = np.asarray(bias, dtype=np.float32)

    nc = build_program()
    in_maps = pack_inputs(xs, weight)
    res = run_bass_kernel_spmd(nc, in_maps, list(range(NCORE)))
    return unpack_outputs(res.results, bias)


# revision 21
# speedup vs baseline: 1.0996x; 1.0996x over previous
"""Trainium2 Bass kernel for DirectionalConvLayer.

Problem: 4 directional 3-tap convs over [256, 256, 15, 15] fp32 images, one
input per direction (horizontal / vertical / main-diagonal / anti-diagonal
taps), shared weight [256, 256, 3] and bias [256].

Strategy: every direction is a 1-D 3-tap conv along its set of lines
(rows / columns / diagonals / anti-diagonals) with a dense 256x256 channel
mix per tap. On the host, ALL lines of ALL four inputs are packed
back-to-back (no separators) into one flat stream, split across 8 cores at
line boundaries. The device kernel is direction-agnostic: a pure 3-tap conv
along the flat axis — accumulating matmuls against +/-1-shifted views of
the stream, contraction over C_in in two 128-chunks. The conv contaminates
the two outputs at every line junction with one known term each; the host
subtracts those (two batched matmuls) during unpacking.

Transfers and matmul operands are float16 (10-bit mantissa, ~ the tensor
engine's own fp32r precision) at full 1-cycle/row PE rate; PSUM accumulates
in fp32. Per core: 62 free-tiles of 512 cols; per tile x 2 cout-chunks:
6 accumulating matmuls (3 taps x 2 cin-chunks), then a vector-engine
PSUM->SBUF cast and DMA out. Bias is added on the host during unpacking.
"""
from contextlib import ExitStack

import numpy as np

import concourse.bass as bass
import concourse.tile as tile
from concourse import mybir
from concourse.bass_utils import run_bass_kernel_spmd

P = 128
FT = 512
NTILE = 57
CORE_COLS = NTILE * FT     # 29184 cols per core
H = W = 15
NCORE = 8

MM_DT = mybir.dt.float16
MM_NP = np.float16


def _build_lines(d):
    if d == 0:
        return [[(i, j) for j in range(W)] for i in range(H)]
    if d == 1:
        return [[(i, j) for i in range(H)] for j in range(W)]
    if d == 2:
        return [
            [(i, i - k) for i in range(max(0, k), min(H, H + k))]
            for k in range(-(W - 1), W)
        ]
    return [
        [(i, s - i) for i in range(max(0, s - (W - 1)), min(H, s + 1))]
        for s in range(H + W - 1)
    ]


def _build_stream_map():
    """Greedy-pack every (direction, image, line) into NCORE x CORE_COLS,
    back-to-back with NO separators. colmap[d, b, i*W+j] = core * CORE_COLS
    + local_col. At every line-to-line junction the device conv contaminates
    the two adjacent outputs (tap w2 of the left line's last cell reads the
    right line's first cell and vice versa); those two known terms are
    subtracted on the host (`_BOUND_STARTS`). Core slices start at line
    starts, so taps at core edges only read the DRAM zero guards."""
    colmap = np.full((4, 256, H * W), -1, np.int64)
    starts = []
    core, col = 0, 0
    for d in range(4):
        lines = _build_lines(d)
        for b in range(256):
            for ln in lines:
                ll = len(ln)
                if col + ll > CORE_COLS:
                    core += 1
                    col = 0
                    assert core < NCORE, "stream overflow"
                if col > 0:
                    starts.append(core * CORE_COLS + col)
                for i, (r, c) in enumerate(ln):
                    colmap[d, b, r * W + c] = core * CORE_COLS + col + i
                col += ll
    assert (colmap >= 0).all()
    return colmap, np.array(starts, np.int64)


_COLMAP, _BOUND_STARTS = _build_stream_map()


def _split_drain_waits(nc, max_waits=1):
    """Workaround for this walrus build's 'Too many sync wait commands' limit
    (1 sync wait per instruction): hoist excess sem-waits onto nop
    instructions inserted right before the instruction on the same engine.
    Sequential waits on one engine queue are equivalent to multiple waits on
    one instruction."""
    n = 0
    for fn in nc.m.functions:
        for bb in fn.blocks:
            insts = bb.instructions
            i = 0
            while i < len(insts):
                inst = insts[i]
                si = inst.sync_info
                if si is not None and si.on_wait and len(si.on_wait) > max_waits:
                    extra = list(si.on_wait)[max_waits:]
                    si.on_wait = list(si.on_wait)[:max_waits]
                    for wt in extra:
                        nop = mybir.InstNoOp(
                            name=f"I-waitsplit-{n}",
                            engine=inst.engine,
                            sync_info=mybir.SyncInfo(on_wait=[wt], on_update=[]),
                        )
                        nc.register_instruction(nop)
                        n += 1
                        insts.insert(i, nop)
                        i += 1
                i += 1
    return n


def build_program():
    nc = bass.Bass("TRN2", target_bir_lowering=False, debug=False, num_devices=8)
    xin = nc.dram_tensor(
        "xin", [P, 2, CORE_COLS + 2], MM_DT, kind="ExternalInput"
    ).ap()
    wts = nc.dram_tensor(
        "wts", [P, 2, 3, 2, 128], MM_DT, kind="ExternalInput"
    ).ap()
    yout = nc.dram_tensor(
        "yout", [P, 2, CORE_COLS], MM_DT, kind="ExternalOutput"
    ).ap()

    # DMA unit sizes in free-tiles: small prologue units so the PE starts
    # quickly, large middle units for DMA efficiency, small epilogue units to
    # shorten the tail.
    units = [1, 1, 2] + [4] * 12 + [2, 2, 1]
    assert sum(units) == NTILE

    with tile.TileContext(nc) as tc, ExitStack() as ctx:
        cpool = ctx.enter_context(tc.tile_pool(name="const", bufs=1))
        xpool = ctx.enter_context(tc.tile_pool(name="x", bufs=4))
        ypool = ctx.enter_context(tc.tile_pool(name="y", bufs=4))
        ppool = ctx.enter_context(tc.tile_pool(name="ps", bufs=8, space="PSUM"))

        # PE warmup: dummy matmuls with no DMA dependency, issued while the
        # first transfers are in flight, so the HAM clock-gate reaches 8/8
        # (2.4 GHz) before the real matmuls start (cold PE runs at 1.2 GHz).
        # ~30 x 128-free matmuls ~= 4 us of PE activity, past the ~3.4 us
        # HAM window.
        warm = cpool.tile([P, P], MM_DT)
        nc.gpsimd.memset(warm[:], 0.0)
        wscr = cpool.tile([P, 1], mybir.dt.float32)
        wps = ppool.tile([P, P], mybir.dt.float32, tag="ps")
        for i in range(30):
            nc.tensor.matmul(
                wps[:], warm[:], warm[:], start=(i == 0), stop=(i == 29)
            )
        nc.vector.tensor_copy(wscr[:], wps[:, :1])

        wt0 = cpool.tile([P, 3, 2, 128], MM_DT)
        nc.gpsimd.dma_start(wt0[:], wts[:, 0])
        wt1 = cpool.tile([P, 3, 2, 128], MM_DT)
        nc.gpsimd.dma_start(wt1[:], wts[:, 1])
        wthalf = (wt0, wt1)

        tile0 = 0
        for u in units:
            ucol = u * FT
            base = tile0 * FT
            xt = xpool.tile([P, 2, ucol + 2], MM_DT)
            nc.sync.dma_start(xt[:], xin[:, :, base : base + ucol + 2])
            yt = ypool.tile([P, 2, ucol], MM_DT)
            for f in range(u):
                for o in range(2):
                    ps = ppool.tile([P, FT], mybir.dt.float32, tag="ps")
                    g = 0
                    for t in range(3):
                        for k in range(2):
                            lhsT = wthalf[o][:, t, k, :]
                            rhs = xt[:, k, f * FT + t : f * FT + t + FT]
                            nc.tensor.matmul(
                                ps[:], lhsT, rhs, start=(g == 0), stop=(g == 5)
                            )
                            g += 1
                    nc.vector.tensor_copy(yt[:, o, f * FT : (f + 1) * FT], ps[:])
            if u == 1:
                # tail units: one DMA per cout half so the first half ships
                # as soon as its cast lands
                nc.sync.dma_start(yout[:, 0, base : base + ucol], yt[:, 0])
                nc.sync.dma_start(yout[:, 1, base : base + ucol], yt[:, 1])
            else:
                nc.sync.dma_start(yout[:, :, base : base + ucol], yt[:])
            tile0 += u
    _split_drain_waits(nc)
    return nc


def pack_inputs(xs, weight):
    """xs: list of 4 arrays [256, 256, 15, 15] fp32. in_maps for cores 0-7."""
    # w_dev[p, o2, t, k, m] = weight[o2*128+m, k*128+p, t]
    w_dev = np.ascontiguousarray(
        weight.reshape(2, P, 2, P, 3)       # [o2, m, k, p, t]
        .transpose(3, 0, 4, 2, 1)           # [p, o2, t, k, m]
    ).astype(MM_NP)

    C = 256
    xflat = np.zeros((C, NCORE * CORE_COLS), MM_NP)
    for d in range(4):
        xflat[:, _COLMAP[d].reshape(-1)] = (
            xs[d].transpose(1, 0, 2, 3).reshape(C, -1).astype(MM_NP)
        )

    in_maps = []
    for core in range(NCORE):
        seg = xflat[:, core * CORE_COLS : (core + 1) * CORE_COLS]
        xin_np = np.zeros((P, 2, CORE_COLS + 2), MM_NP)
        xin_np[:, 0, 1 : CORE_COLS + 1] = seg[:P]
        xin_np[:, 1, 1 : CORE_COLS + 1] = seg[P:]
        in_maps.append({"xin": xin_np, "wts": w_dev})
    return in_maps, xflat


def unpack_outputs(results, bias, weight, xflat):
    O = 256
    yflat = np.empty((O, NCORE * CORE_COLS), np.float32)
    for core in range(NCORE):
        yo = np.asarray(results[core]["yout"])        # [128, 2, CORE_COLS] fp16
        yflat[:, core * CORE_COLS : (core + 1) * CORE_COLS] = (
            yo.transpose(1, 0, 2).reshape(O, CORE_COLS).astype(np.float32)
        )

    # subtract line-junction contamination (fp16-quantized operands match
    # what the device multiplied, so the residual is only PSUM rounding)
    starts = _BOUND_STARTS
    ends = starts - 1
    w0q = weight[:, :, 0].astype(MM_NP).astype(np.float32)
    w2q = weight[:, :, 2].astype(MM_NP).astype(np.float32)
    xs_q = xflat[:, starts].astype(np.float32)
    xe_q = xflat[:, ends].astype(np.float32)
    yflat[:, ends] -= w2q @ xs_q
    yflat[:, starts] -= w0q @ xe_q
    outs = []
    b = bias[None, :, None].astype(np.float32)
    for d in range(4):
        yd = yflat[:, _COLMAP[d].reshape(-1)].reshape(O, 256, H * W)
        yd = yd.transpose(1, 0, 2) + b
        outs.append(np.ascontiguousarray(yd.reshape(256, 256, H, W)))
    return tuple(outs)


def kernel(x0, x1, x2, x3, weight, bias):
    xs = [np.ascontiguousarray(np.asarray(a, dtype=np.float32)) for a in (x0, x1, x2, x3)]
    weight = np.asarray(weight, dtype=np.float32)
    bias = np.asarray(bias, dtype=np.float32)

    nc = build_program()
    in_maps, xflat = pack_inputs(xs, weight)
    res = run_bass_kernel_spmd(nc, in_maps, list(range(NCORE)))
    return unpack_outputs(res.results, bias, weight, xflat)
